# revision 34
# baseline (speedup 1.0000x reference)
"""DGCNN (3x DynamicEdgeConv, kNN=20) Trainium2 Bass kernel.

Self-contained: `kernel(**inputs) -> np.ndarray` takes the full inputs from
setup_inputs() (pos [8,4096,3] + 9 weight/bias pairs) and returns [8,4096,64].

Sharding: data-parallel over batch B=8 -> one point cloud per NeuronCore,
weights replicated. Each core runs the identical program on its slice.

Per-core, per-layer pipeline (N=4096 points, feature dim D in {3,64}, H=64):
  S = (x2t+ones)^T @ rb = 4 x_i.x_j - 2 sq_j in one PE pass (row-monotone in
      -distance; rb carries -2sq in its last row, lhsT a ones row)
  top-24 exact per row: 3 rounds of DVE max8/max_index with match_replace
  h1 = leaky(U_i + V_j): prefill U (ACT), 20x indirect DMA gather with
      CCE-add over 4 SWDGE queues, Prelu
  h1 -> 20 PE transposes -> h1T [64, 20*128]; h2T/h3T = Prelu(W@h + b);
  out tile = max over k (DVE strided reduce)
Emission is software-pipelined (stages lag: gather/MLP by 1 tile, reduce by
2) because engines run their streams in order. Each layer's rb/U/V/-2sq for
the NEXT layer are produced inside the reduce stage (per-tile epilogue), so
layer boundaries cost only a few us.
"""
import numpy as np

import concourse.bass as bass
import concourse.bacc as bacc
import concourse.mybir as mybir
import concourse.tile as tile
from concourse.bass_utils import run_bass_kernel_spmd
from concourse.masks import make_identity

F32 = mybir.dt.float32
U32 = mybir.dt.uint32
AF = mybir.ActivationFunctionType
ALU = mybir.AluOpType

B = 8
N = 4096
P = 128
NT = N // P            # 32 row tiles
K = 20
H = 64
SLOPE = 0.2
NEG = -3.0e38
NSW = 4                # SWDGE queues for the edge gathers

_CACHE = {}

import os
_SKIP = os.environ.get("AMP_SKIP", "")   # ablation: "topk" | "gather" | "dist"


def _gather_q(nc, q, **kw):
    bi = nc.gpsimd.indirect_dma_start(**kw)
    if q:
        bi.ins.queue = f"qPoolDynamic{q}"
    return bi


def _load_weights(nc, sb, li, w1_d, b1_d, w2_d, b2_d, w3_d, b3_d, d_in):
    """Load + prep one layer's weights: wd2 = (W1a-W1b)/2, wb2 = W1b/2."""
    wa = sb.tile([d_in, H], F32, tag=f"wa{li}")
    wb = sb.tile([d_in, H], F32, tag=f"wb{li}")
    nc.sync.dma_start(wa[:], w1_d[0:d_in, :])
    nc.sync.dma_start(wb[:], w1_d[d_in:2 * d_in, :])
    wd2 = sb.tile([d_in, H], F32, tag=f"wd2{li}")
    nc.vector.tensor_tensor(out=wd2[:], in0=wa[:], in1=wb[:], op=ALU.subtract)
    nc.vector.tensor_scalar_mul(wd2[:], wd2[:], 0.5)
    wb2 = sb.tile([d_in, H], F32, tag=f"wb2{li}")
    nc.vector.tensor_scalar_mul(wb2[:], wb[:], 0.5)
    # block-diagonal [[W,0],[0,W]] so one 128-contraction matmul applies the
    # edge MLP to two neighbors at once (both halves of the PE array busy)
    w2d = sb.tile([2 * H, 2 * H], F32, tag=f"w2d{li}")
    nc.vector.memset(w2d[:], 0.0)
    nc.sync.dma_start(w2d[0:H, 0:H], w2_d[:])
    nc.sync.dma_start(w2d[H:2 * H, H:2 * H], w2_d[:])
    w3d = sb.tile([2 * H, 2 * H], F32, tag=f"w3d{li}")
    nc.vector.memset(w3d[:], 0.0)
    nc.sync.dma_start(w3d[0:H, 0:H], w3_d[:])
    nc.sync.dma_start(w3d[H:2 * H, H:2 * H], w3_d[:])
    b1r = sb.tile([1, H], F32, tag=f"b1r{li}")
    nc.sync.dma_start(b1r[:], b1_d[:].unsqueeze(0))
    b2s = sb.tile([2 * H, 1], F32, tag=f"b2s{li}")
    nc.sync.dma_start(b2s[0:H, :], b2_d[:].unsqueeze(1))
    nc.sync.dma_start(b2s[H:2 * H, :], b2_d[:].unsqueeze(1))
    b3s = sb.tile([2 * H, 1], F32, tag=f"b3s{li}")
    nc.sync.dma_start(b3s[0:H, :], b3_d[:].unsqueeze(1))
    nc.sync.dma_start(b3s[H:2 * H, :], b3_d[:].unsqueeze(1))
    return dict(wd2=wd2, wb2=wb2, w2d=w2d, w3d=w3d, b1r=b1r, b2s=b2s, b3s=b3s)


def _emit_uv_tile(nc, g, t, x2t, d_in, W, u_sb, vbuf):
    """U/V for one 128-point tile: U = x@(W1a-W1b)+b1, V = x@W1b (from 2X^T)."""
    pp_tp = g["pp_tp"]
    lhs = x2t[0:d_in, t * P:(t + 1) * P]
    pu = pp_tp.tile([P, H], F32, tag="tp")
    nc.tensor.matmul(out=pu[:], lhsT=lhs, rhs=W["wd2"][:], start=True, stop=False)
    nc.tensor.matmul(out=pu[:], lhsT=g["ones1"][:, 0:P], rhs=W["b1r"][:],
                     start=False, stop=True)
    nc.scalar.copy(out=u_sb[:, t * H:(t + 1) * H], in_=pu[:])
    pv = pp_tp.tile([P, H], F32, tag="tp")
    nc.tensor.matmul(out=pv[:], lhsT=lhs, rhs=W["wb2"][:], start=True, stop=True)
    nc.scalar.copy(out=vbuf[:, t * H:(t + 1) * H], in_=pv[:])


def _build_layer(nc, g, d_in, x2t, rb, u_sb, W, v_d, nxt, out_d=None):
    """Emit one EdgeConv layer.

    x2t: 2*X^T rows 0..d_in-1 + ones in row d_in. rb: same rows + -2sq in row
    d_in. u_sb: U point-major. v_d: V table in DRAM. nxt (non-final layers):
    dict(x2t, rb, u_sb, vbuf, v_d, W) -- the epilogue of each reduce stage
    produces the next layer's inputs per tile. out_d: final output (layer 3).
    """
    sb = g["sb"]
    pp_s, pp_tp, pp_h = g["pp_s"], g["pp_tp"], g["pp_h"]
    ident, ones1, alpha64, alpha128 = (
        g["ident"], g["ones1"], g["alpha64"], g["alpha128"])
    onescol = g["onescol"]

    idxts = {}
    h1s = {}
    h3ts = {}

    def stage_dist_topk(t):
        s_sb = g["s_pool"].tile([P, N], F32, tag="s")
        if _SKIP != "dist":
            for c in range(N // 512):
                ps = pp_s.tile([P, 512], F32, tag="dist")
                nc.tensor.matmul(out=ps[:],
                                 lhsT=x2t[0:d_in + 1, t * P:(t + 1) * P],
                                 rhs=rb[0:d_in + 1, c * 512:(c + 1) * 512],
                                 start=True, stop=True)
                nc.scalar.copy(out=s_sb[:, c * 512:(c + 1) * 512], in_=ps[:])

        winners = g["k_pool"].tile([P, 24], F32, tag="win")
        idxt = g["k_pool"].tile([P, 24], U32, tag="idxt")
        if _SKIP == "topk":
            nc.vector.memset(idxt[:], 0)
        else:
            for r in range(3):
                nc.vector.max(out=winners[:, r * 8:(r + 1) * 8], in_=s_sb[:])
                nc.vector.max_index(out=idxt[:, r * 8:(r + 1) * 8],
                                    in_max=winners[:, r * 8:(r + 1) * 8],
                                    in_values=s_sb[:])
                if r < 2:
                    nc.vector.match_replace(out=s_sb[:],
                                            in_to_replace=winners[:, r * 8:(r + 1) * 8],
                                            in_values=s_sb[:], imm_value=NEG)
        idxts[t] = idxt
        # prefill h1 with U_i now -- it only needs u_sb, so the CCE-add
        # gathers can fire the moment idxt lands
        h1 = g["h1_pool"].tile([P, K * H], F32, tag="h1")
        nc.scalar.copy(out=h1[:].rearrange("p (k f) -> p k f", k=K),
                       in_=u_sb[:, t * H:(t + 1) * H].unsqueeze(1).to_broadcast([P, K, H]))
        h1s[t] = h1

    def stage_gather_mlp(t):
        idxt = idxts.pop(t)
        h1 = h1s.pop(t)
        for k in (range(0) if _SKIP == "gather" else range(K)):
            _gather_q(nc, k % NSW,
                      out=h1[:, k * H:(k + 1) * H], out_offset=None,
                      in_=v_d[:],
                      in_offset=bass.IndirectOffsetOnAxis(ap=idxt[:, k:k + 1], axis=0),
                      compute_op=ALU.add)
        nc.scalar.activation(out=h1[:], in_=h1[:], func=AF.Prelu, alpha=alpha128[:])

        # transpose PAIRS of neighbors: [128,128] blocks -> h1t2 rows 0..63 =
        # even k's features, rows 64..127 = odd k's (for the block-diag MLP)
        KP = K // 2                       # 10 neighbor pairs
        h1t = g["ht_pool"].tile([2 * H, KP * P], F32, tag="ht")
        for off, cnt in ((0, 4), (4, 4), (8, 2)):
            pt = pp_tp.tile([P, 512], F32, tag="tp")
            for j in range(cnt):
                k2 = off + j
                nc.tensor.transpose(out=pt[:, j * P:(j + 1) * P],
                                    in_=h1[:, k2 * 2 * H:(k2 + 1) * 2 * H],
                                    identity=ident[:])
            nc.scalar.copy(out=h1t[:, off * P:(off + cnt) * P],
                           in_=pt[:, 0:cnt * P])

        chunks = ((0, 512), (512, 512), (1024, 256))
        h2t = g["ht_pool"].tile([2 * H, KP * P], F32, tag="ht")
        for c0, cw in chunks:
            ph = pp_h.tile([P, 512], F32, tag="h")
            nc.tensor.matmul(out=ph[:, 0:cw], lhsT=W["w2d"][:],
                             rhs=h1t[:, c0:c0 + cw],
                             start=True, stop=True)
            nc.scalar.activation(out=h2t[:, c0:c0 + cw], in_=ph[:, 0:cw],
                                 func=AF.Prelu, bias=W["b2s"][:], alpha=alpha128[:])
        h3t = g["ht_pool"].tile([2 * H, KP * P], F32, tag="ht")
        for c0, cw in chunks:
            ph = pp_h.tile([P, 512], F32, tag="h")
            nc.tensor.matmul(out=ph[:, 0:cw], lhsT=W["w3d"][:],
                             rhs=h2t[:, c0:c0 + cw],
                             start=True, stop=True)
            nc.scalar.activation(out=h3t[:, c0:c0 + cw], in_=ph[:, 0:cw],
                                 func=AF.Prelu, bias=W["b3s"][:], alpha=alpha128[:])
        h3ts[t] = h3t

    def stage_reduce_out(t):
        h3t = h3ts.pop(t)
        # max over the 10 pair-blocks -> r128 (even k's on rows 0..63, odd
        # k's on rows 64..127); DVE cannot combine different partition
        # starts, so transpose to point-major and fold halves as free slices
        r128 = g["f_pool"].tile([P, P], F32, tag="r128")
        nc.vector.tensor_reduce(out=r128[:],
                                in_=h3t[:].rearrange("h (k i) -> h i k", k=K // 2),
                                axis=mybir.AxisListType.X, op=ALU.max)
        pr = pp_tp.tile([P, P], F32, tag="tp")
        nc.tensor.transpose(out=pr[:], in_=r128[:], identity=ident[:])
        rT = g["f_pool"].tile([P, P], F32, tag="rT")
        nc.scalar.copy(out=rT[:], in_=pr[:])
        ftileT = g["f_pool"].tile([P, H], F32, tag="fT")   # [point, feature]
        nc.vector.tensor_tensor(out=ftileT[:], in0=rT[:, 0:H],
                                in1=rT[:, H:2 * H], op=ALU.max)
        cols = slice(t * P, (t + 1) * P)
        if out_d is None:
            pf = pp_tp.tile([H, P], F32, tag="tp")
            nc.tensor.transpose(out=pf[:], in_=ftileT[:], identity=ident[:])
            # next layer's 2*X^T and rb data rows
            nc.scalar.mul(out=nxt["x2t"][0:H, cols], in_=pf[:], mul=2.0)
            nc.scalar.mul(out=nxt["rb"][0:H, cols], in_=pf[:], mul=2.0)
            # -2*||x||^2 for this tile -> rb's row H (via partition-0 staging
            # + DMA; engines cannot write single partitions at offset H)
            sqt = g["sq_pool"].tile([H, P], F32, tag="sq")
            nc.scalar.activation(out=sqt[:], in_=pf[:], func=AF.Square)
            pq = pp_tp.tile([1, P], F32, tag="tp")
            nc.tensor.matmul(out=pq[:], lhsT=onescol[:], rhs=sqt[:],
                             start=True, stop=True)
            nq = g["nq_pool"].tile([1, P], F32, tag="nq")
            nc.scalar.activation(out=nq[:], in_=pq[:], func=AF.Copy, scale=-2.0)
            nc.sync.dma_start(nxt["rb"][H:H + 1, cols], nq[:])
            # next layer's U and V for this tile
            _emit_uv_tile(nc, g, t, nxt["x2t"], H, nxt["W"], nxt["u_sb"],
                          nxt["vbuf"])
        else:
            nc.scalar.copy(out=g["obuf"][:, t * H:(t + 1) * H], in_=ftileT[:])

    for t in range(NT + 2):
        if t < NT:
            stage_dist_topk(t)
        if 1 <= t <= NT:
            stage_gather_mlp(t - 1)
        if t >= 2:
            stage_reduce_out(t - 2)

    if out_d is None:
        nc.sync.dma_start(nxt["v_d"][:].rearrange("(t p) f -> p t f", p=P),
                          nxt["vbuf"][:].rearrange("p (t f) -> p t f", f=H))
    else:
        # first half already final after tile 15's reduce -- ship it early so
        # only the second half sits in the drain tail
        hN = NT // 2
        nc.sync.dma_start(
            out_d[0:hN * P, :].rearrange("(t p) f -> p t f", p=P),
            g["obuf"][:, 0:hN * H].rearrange("p (t f) -> p t f", f=H))
        nc.sync.dma_start(
            out_d[hN * P:N, :].rearrange("(t p) f -> p t f", p=P),
            g["obuf"][:, hN * H:NT * H].rearrange("p (t f) -> p t f", f=H))


def build(reps=1):
    nc = bacc.Bacc("TRN2", target_bir_lowering=False, debug=False,
                   num_swdge_queues=NSW)
    pos_d = nc.dram_tensor("pos", [N, 3], F32, kind="ExternalInput")
    wnames = {}
    for li, pfx in enumerate(("1", "2", "3")):
        d2 = 6 if li == 0 else 128
        wnames[f"w{pfx}1"] = nc.dram_tensor(f"w{pfx}1", [d2, H], F32, kind="ExternalInput")
        wnames[f"w{pfx}2"] = nc.dram_tensor(f"w{pfx}2", [H, H], F32, kind="ExternalInput")
        wnames[f"w{pfx}3"] = nc.dram_tensor(f"w{pfx}3", [H, H], F32, kind="ExternalInput")
        for j in ("1", "2", "3"):
            wnames[f"b{pfx}{j}"] = nc.dram_tensor(f"b{pfx}{j}", [H], F32, kind="ExternalInput")
    out_d = nc.dram_tensor("out", [N, H], F32, kind="ExternalOutput")
    v_ds = [nc.dram_tensor(f"vtab{li}", [N, H], F32) for li in range(3)]

    with tile.TileContext(nc) as tc:
        with tc.tile_pool(name="sb", bufs=1) as sb, \
             tc.tile_pool(name="s_pool", bufs=3) as s_pool, \
             tc.tile_pool(name="k_pool", bufs=3) as k_pool, \
             tc.tile_pool(name="h1_pool", bufs=3) as h1_pool, \
             tc.tile_pool(name="ht_pool", bufs=5) as ht_pool, \
             tc.tile_pool(name="f_pool", bufs=2) as f_pool, \
             tc.tile_pool(name="sq_pool", bufs=2) as sq_pool, \
             tc.tile_pool(name="nq_pool", bufs=2) as nq_pool, \
             tc.tile_pool(name="pp_s", bufs=2, space="PSUM") as pp_s, \
             tc.tile_pool(name="pp_tp", bufs=2, space="PSUM") as pp_tp, \
             tc.tile_pool(name="pp_h", bufs=4, space="PSUM") as pp_h:

            g = dict(sb=sb, s_pool=s_pool, k_pool=k_pool, h1_pool=h1_pool,
                     ht_pool=ht_pool, f_pool=f_pool, sq_pool=sq_pool,
                     nq_pool=nq_pool, pp_s=pp_s, pp_tp=pp_tp, pp_h=pp_h)

            ident = sb.tile([P, P], F32, tag="ident")
            make_identity(nc, ident)
            g["ident"] = ident
            ones1 = sb.tile([1, P], F32, tag="ones1")
            nc.vector.memset(ones1[:], 1.0)
            g["ones1"] = ones1
            alpha64 = sb.tile([H, 1], F32, tag="alpha64")
            nc.vector.memset(alpha64[:], SLOPE)
            g["alpha64"] = alpha64
            alpha128 = sb.tile([P, 1], F32, tag="alpha128")
            nc.vector.memset(alpha128[:], SLOPE)
            g["alpha128"] = alpha128
            onescol = sb.tile([H, 1], F32, tag="onescol")
            nc.vector.memset(onescol[:], 1.0)
            g["onescol"] = onescol

            # all layers' weights up front
            Ws = [
                _load_weights(nc, sb, 0, wnames["w11"], wnames["b11"],
                              wnames["w12"], wnames["b12"], wnames["w13"],
                              wnames["b13"], 3),
                _load_weights(nc, sb, 1, wnames["w21"], wnames["b21"],
                              wnames["w22"], wnames["b22"], wnames["w23"],
                              wnames["b23"], H),
                _load_weights(nc, sb, 2, wnames["w31"], wnames["b31"],
                              wnames["w32"], wnames["b32"], wnames["w33"],
                              wnames["b33"], H),
            ]

            # x2t / rb ping-pong ([H+1, N]: data rows + ones / -2sq row).
            # memset to 1.0 so row d_in serves as the ones row (never
            # rewritten; layer 1 uses row 3, layers 2/3 row 64).
            x2t_a = sb.tile([H + 1, N], F32, tag="x2t_a")
            x2t_b = sb.tile([H + 1, N], F32, tag="x2t_b")
            rb_a = sb.tile([H + 1, N], F32, tag="rb_a")
            rb_b = sb.tile([H + 1, N], F32, tag="rb_b")
            u_a = sb.tile([P, NT * H], F32, tag="u_a")
            u_b = sb.tile([P, NT * H], F32, tag="u_b")
            vbuf = sb.tile([P, NT * H], F32, tag="vbuf")
            g["obuf"] = vbuf   # final layer stages its output here
            xsb = sb.tile([P, NT * 3], F32, tag="xsb")

            for _rep in range(reps):
                nc.vector.memset(x2t_a[:], 1.0)
                nc.vector.memset(x2t_b[:], 1.0)

                # ---- init: pos -> 2*X^T rows 0..2 of x2t_a ----
                nc.sync.dma_start(xsb[:].rearrange("p (t d) -> p t d", d=3),
                                  pos_d[:].rearrange("(t p) d -> p t d", p=P))
                for t in range(NT):
                    pt = pp_tp.tile([3, P], F32, tag="tp")
                    nc.tensor.transpose(out=pt[:], in_=xsb[:, t * 3:(t + 1) * 3],
                                        identity=ident[:])
                    nc.scalar.mul(out=x2t_a[0:3, t * P:(t + 1) * P], in_=pt[:], mul=2.0)

                # ---- init: rb_a rows 0..2 + -2sq row 3; U/V for layer 1 ----
                nc.scalar.copy(out=rb_a[0:3, :], in_=x2t_a[0:3, :])
                xsq = s_pool.tile([P, N], F32, tag="s")   # borrow an S buffer
                nc.scalar.activation(out=xsq[0:3, :], in_=x2t_a[0:3, :], func=AF.Square)
                nrow = s_pool.tile([P, N], F32, tag="s")  # borrow: row 0 stages -2sq
                for c in range(N // 512):
                    ps = pp_tp.tile([1, 512], F32, tag="tp")
                    nc.tensor.matmul(out=ps[:], lhsT=onescol[0:3, :],
                                     rhs=xsq[0:3, c * 512:(c + 1) * 512],
                                     start=True, stop=True)
                    # ps holds 4*sq (x2t holds 2X); write -2sq
                    nc.scalar.activation(out=nrow[0:1, c * 512:(c + 1) * 512],
                                         in_=ps[:], func=AF.Copy, scale=-0.5)
                nc.sync.dma_start(rb_a[3:4, :], nrow[0:1, :])
                for t in range(NT):
                    _emit_uv_tile(nc, g, t, x2t_a, 3, Ws[0], u_a, vbuf)
                nc.sync.dma_start(v_ds[0][:].rearrange("(t p) f -> p t f", p=P),
                                  vbuf[:].rearrange("p (t f) -> p t f", f=H))

                _build_layer(nc, g, 3, x2t_a, rb_a, u_a, Ws[0], v_ds[0],
                             nxt=dict(x2t=x2t_b, rb=rb_b, u_sb=u_b, vbuf=vbuf,
                                      v_d=v_ds[1], W=Ws[1]))
                _build_layer(nc, g, H, x2t_b, rb_b, u_b, Ws[1], v_ds[1],
                             nxt=dict(x2t=x2t_a, rb=rb_a, u_sb=u_a, vbuf=vbuf,
                                      v_d=v_ds[2], W=Ws[2]))
                _build_layer(nc, g, H, x2t_a, rb_a, u_a, Ws[2], v_ds[2], nxt=None,
                             out_d=out_d)
    nc.finalize()
    return nc


def kernel(**inputs):
    if "nc" not in _CACHE:
        _CACHE["nc"] = build()
    nc = _CACHE["nc"]
    pos = np.ascontiguousarray(np.asarray(inputs["pos"], dtype=np.float32))
    weights = {k: np.ascontiguousarray(np.asarray(v, dtype=np.float32))
               for k, v in inputs.items() if k != "pos"}
    in_maps = []
    for b in range(B):
        m = {"pos": pos[b]}
        m.update(weights)
        in_maps.append(m)
    res = run_bass_kernel_spmd(nc, in_maps, core_ids=list(range(B)))
    out = np.stack([res.results[b]["out"] for b in range(B)], axis=0)
    return out


if __name__ == "__main__":
    rng = np.random.default_rng(0)
    fake = {"pos": rng.standard_normal((B, N, 3)).astype(np.float32)}
    for pfx in ("1", "2", "3"):
        d2 = 6 if pfx == "1" else 128
        fake[f"w{pfx}1"] = rng.standard_normal((d2, H)).astype(np.float32) * 0.2
        fake[f"w{pfx}2"] = rng.standard_normal((H, H)).astype(np.float32) * 0.12
        fake[f"w{pfx}3"] = rng.standard_normal((H, H)).astype(np.float32) * 0.12
        for j in ("1", "2", "3"):
            fake[f"b{pfx}{j}"] = np.zeros(H, np.float32)
    o = kernel(**fake)
    print("out", o.shape, o.dtype, float(np.abs(o).max()))


# revision 47
# speedup vs baseline: 1.0547x; 1.0547x over previous
"""DGCNN (3x DynamicEdgeConv, kNN=20) Trainium2 Bass kernel.

Self-contained: `kernel(**inputs) -> np.ndarray` takes the full inputs from
setup_inputs() (pos [8,4096,3] + 9 weight/bias pairs) and returns [8,4096,64].

Sharding: data-parallel over batch B=8 -> one point cloud per NeuronCore,
weights replicated. Each core runs the identical program on its slice.

Per-core, per-layer pipeline (N=4096 points, feature dim D in {3,64}, H=64):
  S = (x2t+ones)^T @ rb = 4 x_i.x_j - 2 sq_j in one PE pass (row-monotone in
      -distance; rb carries -2sq in its last row, lhsT a ones row)
  top-24 exact per row: 3 rounds of DVE max8/max_index with match_replace
  h1 = leaky(U_i + V_j): prefill U (ACT), 20x indirect DMA gather with
      CCE-add over 4 SWDGE queues, Prelu
  h1 -> 20 PE transposes -> h1T [64, 20*128]; h2T/h3T = Prelu(W@h + b);
  out tile = max over k (DVE strided reduce)
Emission is software-pipelined (stages lag: gather/MLP by 1 tile, reduce by
2) because engines run their streams in order. Each layer's rb/U/V/-2sq for
the NEXT layer are produced inside the reduce stage (per-tile epilogue), so
layer boundaries cost only a few us.
"""
import numpy as np

import concourse.bass as bass
import concourse.bacc as bacc
import concourse.mybir as mybir
import concourse.tile as tile
from concourse.bass_utils import run_bass_kernel_spmd
from concourse.masks import make_identity

F32 = mybir.dt.float32
U32 = mybir.dt.uint32
AF = mybir.ActivationFunctionType
ALU = mybir.AluOpType

B = 8
N = 4096
P = 128
NT = N // P            # 32 row tiles
K = 20
H = 64
SLOPE = 0.2
NEG = -3.0e38
NSW = 4                # SWDGE queues for the edge gathers

_CACHE = {}

import os
_SKIP = os.environ.get("AMP_SKIP", "")   # ablation: "topk" | "gather" | "dist"


def _gather_q(nc, q, **kw):
    bi = nc.gpsimd.indirect_dma_start(**kw)
    if q:
        bi.ins.queue = f"qPoolDynamic{q}"
    return bi


def _load_weights(nc, sb, li, w1_d, b1_d, w2_d, b2_d, w3_d, b3_d, d_in):
    """Load + prep one layer's weights: wd2 = (W1a-W1b)/2, wb2 = W1b/2."""
    wa = sb.tile([d_in, H], F32, tag=f"wa{li}")
    wb = sb.tile([d_in, H], F32, tag=f"wb{li}")
    nc.sync.dma_start(wa[:], w1_d[0:d_in, :])
    nc.sync.dma_start(wb[:], w1_d[d_in:2 * d_in, :])
    wd2 = sb.tile([d_in, H], F32, tag=f"wd2{li}")
    nc.vector.tensor_tensor(out=wd2[:], in0=wa[:], in1=wb[:], op=ALU.subtract)
    nc.vector.tensor_scalar_mul(wd2[:], wd2[:], 0.5)
    wb2 = sb.tile([d_in, H], F32, tag=f"wb2{li}")
    nc.vector.tensor_scalar_mul(wb2[:], wb[:], 0.5)
    # block-diagonal [[W,0],[0,W]] so one 128-contraction matmul applies the
    # edge MLP to two neighbors at once (both halves of the PE array busy)
    w2d = sb.tile([2 * H, 2 * H], F32, tag=f"w2d{li}")
    nc.vector.memset(w2d[:], 0.0)
    nc.sync.dma_start(w2d[0:H, 0:H], w2_d[:])
    nc.sync.dma_start(w2d[H:2 * H, H:2 * H], w2_d[:])
    w3d = sb.tile([2 * H, 2 * H], F32, tag=f"w3d{li}")
    nc.vector.memset(w3d[:], 0.0)
    nc.sync.dma_start(w3d[0:H, 0:H], w3_d[:])
    nc.sync.dma_start(w3d[H:2 * H, H:2 * H], w3_d[:])
    b1r = sb.tile([1, H], F32, tag=f"b1r{li}")
    nc.sync.dma_start(b1r[:], b1_d[:].unsqueeze(0))
    b2s = sb.tile([2 * H, 1], F32, tag=f"b2s{li}")
    nc.sync.dma_start(b2s[0:H, :], b2_d[:].unsqueeze(1))
    nc.sync.dma_start(b2s[H:2 * H, :], b2_d[:].unsqueeze(1))
    b3s = sb.tile([2 * H, 1], F32, tag=f"b3s{li}")
    nc.sync.dma_start(b3s[0:H, :], b3_d[:].unsqueeze(1))
    nc.sync.dma_start(b3s[H:2 * H, :], b3_d[:].unsqueeze(1))
    return dict(wd2=wd2, wb2=wb2, w2d=w2d, w3d=w3d, b1r=b1r, b2s=b2s, b3s=b3s)


def _emit_uv_tile(nc, g, t, x2t, d_in, W, u_sb, vbuf):
    """U/V for one 128-point tile: U = x@(W1a-W1b)+b1, V = x@W1b (from 2X^T)."""
    pp_tp = g["pp_tp"]
    lhs = x2t[0:d_in, t * P:(t + 1) * P]
    pu = pp_tp.tile([P, H], F32, tag="tp")
    nc.tensor.matmul(out=pu[:], lhsT=lhs, rhs=W["wd2"][:], start=True, stop=False)
    nc.tensor.matmul(out=pu[:], lhsT=g["ones1"][:, 0:P], rhs=W["b1r"][:],
                     start=False, stop=True)
    nc.scalar.copy(out=u_sb[:, t * H:(t + 1) * H], in_=pu[:])
    pv = pp_tp.tile([P, H], F32, tag="tp")
    nc.tensor.matmul(out=pv[:], lhsT=lhs, rhs=W["wb2"][:], start=True, stop=True)
    nc.scalar.copy(out=vbuf[:, t * H:(t + 1) * H], in_=pv[:])


def _build_layer(nc, g, d_in, x2t, rb, u_sb, W, v_d, nxt, out_d=None):
    """Emit one EdgeConv layer.

    x2t: 2*X^T rows 0..d_in-1 + ones in row d_in. rb: same rows + -2sq in row
    d_in. u_sb: U point-major. v_d: V table in DRAM. nxt (non-final layers):
    dict(x2t, rb, u_sb, vbuf, v_d, W) -- the epilogue of each reduce stage
    produces the next layer's inputs per tile. out_d: final output (layer 3).
    """
    sb = g["sb"]
    pp_s, pp_tp, pp_h = g["pp_s"], g["pp_tp"], g["pp_h"]
    ident, ones1, alpha64, alpha128 = (
        g["ident"], g["ones1"], g["alpha64"], g["alpha128"])
    onescol = g["onescol"]

    idxts = {}
    h1s = {}
    h3ts = {}

    def stage_dist_topk(t):
        s_sb = g["s_pool"].tile([P, N], F32, tag="s")
        if _SKIP != "dist":
            for c in range(N // 512):
                ps = pp_s.tile([P, 512], F32, tag="dist")
                nc.tensor.matmul(out=ps[:],
                                 lhsT=x2t[0:d_in + 1, t * P:(t + 1) * P],
                                 rhs=rb[0:d_in + 1, c * 512:(c + 1) * 512],
                                 start=True, stop=True)
                nc.scalar.copy(out=s_sb[:, c * 512:(c + 1) * 512], in_=ps[:])

        winners = g["k_pool"].tile([P, 24], F32, tag="win")
        idxt = g["k_pool"].tile([P, 24], U32, tag="idxt")
        if _SKIP == "topk":
            nc.vector.memset(idxt[:], 0)
        else:
            for r in range(3):
                nc.vector.max(out=winners[:, r * 8:(r + 1) * 8], in_=s_sb[:])
                nc.vector.max_index(out=idxt[:, r * 8:(r + 1) * 8],
                                    in_max=winners[:, r * 8:(r + 1) * 8],
                                    in_values=s_sb[:])
                if r < 2:
                    nc.vector.match_replace(out=s_sb[:],
                                            in_to_replace=winners[:, r * 8:(r + 1) * 8],
                                            in_values=s_sb[:], imm_value=NEG)
        idxts[t] = idxt
        # prefill h1 with U_i now -- it only needs u_sb, so the CCE-add
        # gathers can fire the moment idxt lands
        h1 = g["h1_pool"].tile([P, K * H], F32, tag="h1")
        nc.scalar.copy(out=h1[:].rearrange("p (k f) -> p k f", k=K),
                       in_=u_sb[:, t * H:(t + 1) * H].unsqueeze(1).to_broadcast([P, K, H]))
        h1s[t] = h1

    def stage_gather_mlp(t):
        idxt = idxts.pop(t)
        h1 = h1s.pop(t)
        for k in (range(0) if _SKIP == "gather" else range(K)):
            _gather_q(nc, k % NSW,
                      out=h1[:, k * H:(k + 1) * H], out_offset=None,
                      in_=v_d[:],
                      in_offset=bass.IndirectOffsetOnAxis(ap=idxt[:, k:k + 1], axis=0),
                      compute_op=ALU.add)
        nc.scalar.activation(out=h1[:], in_=h1[:], func=AF.Prelu, alpha=alpha128[:])

        # transpose PAIRS of neighbors: [128,128] blocks -> h1t2 rows 0..63 =
        # even k's features, rows 64..127 = odd k's (for the block-diag MLP)
        KP = K // 2                       # 10 neighbor pairs
        h1t = g["ht_pool"].tile([2 * H, KP * P], F32, tag="ht")
        for off, cnt in ((0, 4), (4, 4), (8, 2)):
            pt = pp_tp.tile([P, 512], F32, tag="tp")
            for j in range(cnt):
                k2 = off + j
                nc.tensor.transpose(out=pt[:, j * P:(j + 1) * P],
                                    in_=h1[:, k2 * 2 * H:(k2 + 1) * 2 * H],
                                    identity=ident[:])
            nc.scalar.copy(out=h1t[:, off * P:(off + cnt) * P],
                           in_=pt[:, 0:cnt * P])

        chunks = ((0, 512), (512, 512), (1024, 256))
        h2t = g["ht_pool"].tile([2 * H, KP * P], F32, tag="ht")
        for c0, cw in chunks:
            ph = pp_h.tile([P, 512], F32, tag="h")
            nc.tensor.matmul(out=ph[:, 0:cw], lhsT=W["w2d"][:],
                             rhs=h1t[:, c0:c0 + cw],
                             start=True, stop=True)
            nc.scalar.activation(out=h2t[:, c0:c0 + cw], in_=ph[:, 0:cw],
                                 func=AF.Prelu, bias=W["b2s"][:], alpha=alpha128[:])
        h3t = g["ht_pool"].tile([2 * H, KP * P], F32, tag="ht")
        for c0, cw in chunks:
            ph = pp_h.tile([P, 512], F32, tag="h")
            nc.tensor.matmul(out=ph[:, 0:cw], lhsT=W["w3d"][:],
                             rhs=h2t[:, c0:c0 + cw],
                             start=True, stop=True)
            nc.scalar.activation(out=h3t[:, c0:c0 + cw], in_=ph[:, 0:cw],
                                 func=AF.Prelu, bias=W["b3s"][:], alpha=alpha128[:])
        h3ts[t] = h3t

    def stage_reduce_out(t):
        h3t = h3ts.pop(t)
        # max over the 10 pair-blocks -> r128 (even k's on rows 0..63, odd
        # k's on rows 64..127); DVE cannot combine different partition
        # starts, so transpose to point-major and fold halves as free slices
        r128 = g["f_pool"].tile([P, P], F32, tag="r128")
        nc.vector.tensor_reduce(out=r128[:],
                                in_=h3t[:].rearrange("h (k i) -> h i k", k=K // 2),
                                axis=mybir.AxisListType.X, op=ALU.max)
        pr = pp_tp.tile([P, P], F32, tag="tp")
        nc.tensor.transpose(out=pr[:], in_=r128[:], identity=ident[:])
        rT = g["f_pool"].tile([P, P], F32, tag="rT")
        nc.scalar.copy(out=rT[:], in_=pr[:])
        ftileT = g["f_pool"].tile([P, H], F32, tag="fT")   # [point, feature]
        nc.vector.tensor_tensor(out=ftileT[:], in0=rT[:, 0:H],
                                in1=rT[:, H:2 * H], op=ALU.max)
        cols = slice(t * P, (t + 1) * P)
        if out_d is None:
            pf = pp_tp.tile([H, P], F32, tag="tp")
            nc.tensor.transpose(out=pf[:], in_=ftileT[:], identity=ident[:])
            # next layer's 2*X^T and rb data rows
            nc.scalar.mul(out=nxt["x2t"][0:H, cols], in_=pf[:], mul=2.0)
            nc.scalar.mul(out=nxt["rb"][0:H, cols], in_=pf[:], mul=2.0)
            # -2*||x||^2 for this tile -> rb's row H (via partition-0 staging
            # + DMA; engines cannot write single partitions at offset H)
            sqt = g["sq_pool"].tile([H, P], F32, tag="sq")
            nc.scalar.activation(out=sqt[:], in_=pf[:], func=AF.Square)
            pq = pp_tp.tile([1, P], F32, tag="tp")
            nc.tensor.matmul(out=pq[:], lhsT=onescol[:], rhs=sqt[:],
                             start=True, stop=True)
            nq = g["nq_pool"].tile([1, P], F32, tag="nq")
            nc.scalar.activation(out=nq[:], in_=pq[:], func=AF.Copy, scale=-2.0)
            nc.sync.dma_start(nxt["rb"][H:H + 1, cols], nq[:])
            # next layer's U and V for this tile
            _emit_uv_tile(nc, g, t, nxt["x2t"], H, nxt["W"], nxt["u_sb"],
                          nxt["vbuf"])
        else:
            nc.scalar.copy(out=g["obuf"][:, t * H:(t + 1) * H], in_=ftileT[:])

    for t in range(NT + 2):
        if t < NT:
            stage_dist_topk(t)
        if 1 <= t <= NT:
            stage_gather_mlp(t - 1)
        if t >= 2:
            stage_reduce_out(t - 2)

    if out_d is None:
        nc.sync.dma_start(nxt["v_d"][:].rearrange("(t p) f -> p t f", p=P),
                          nxt["vbuf"][:].rearrange("p (t f) -> p t f", f=H))
    else:
        # first half already final after tile 15's reduce -- ship it early so
        # only the second half sits in the drain tail
        hN = NT // 2
        nc.sync.dma_start(
            out_d[0:hN * P, :].rearrange("(t p) f -> p t f", p=P),
            g["obuf"][:, 0:hN * H].rearrange("p (t f) -> p t f", f=H))
        nc.sync.dma_start(
            out_d[hN * P:N, :].rearrange("(t p) f -> p t f", p=P),
            g["obuf"][:, hN * H:NT * H].rearrange("p (t f) -> p t f", f=H))


def build(reps=1):
    nc = bacc.Bacc("TRN2", target_bir_lowering=False, debug=False,
                   num_swdge_queues=NSW)
    pos_d = nc.dram_tensor("pos", [N, 3], F32, kind="ExternalInput")
    wnames = {}
    for li, pfx in enumerate(("1", "2", "3")):
        d2 = 6 if li == 0 else 128
        wnames[f"w{pfx}1"] = nc.dram_tensor(f"w{pfx}1", [d2, H], F32, kind="ExternalInput")
        wnames[f"w{pfx}2"] = nc.dram_tensor(f"w{pfx}2", [H, H], F32, kind="ExternalInput")
        wnames[f"w{pfx}3"] = nc.dram_tensor(f"w{pfx}3", [H, H], F32, kind="ExternalInput")
        for j in ("1", "2", "3"):
            wnames[f"b{pfx}{j}"] = nc.dram_tensor(f"b{pfx}{j}", [H], F32, kind="ExternalInput")
    out_d = nc.dram_tensor("out", [N, H], F32, kind="ExternalOutput")
    v_ds = [nc.dram_tensor(f"vtab{li}", [N, H], F32) for li in range(3)]

    with tile.TileContext(nc) as tc:
        with tc.tile_pool(name="sb", bufs=1) as sb, \
             tc.tile_pool(name="s_pool", bufs=3) as s_pool, \
             tc.tile_pool(name="k_pool", bufs=3) as k_pool, \
             tc.tile_pool(name="h1_pool", bufs=3) as h1_pool, \
             tc.tile_pool(name="ht_pool", bufs=5) as ht_pool, \
             tc.tile_pool(name="f_pool", bufs=2) as f_pool, \
             tc.tile_pool(name="sq_pool", bufs=2) as sq_pool, \
             tc.tile_pool(name="nq_pool", bufs=2) as nq_pool, \
             tc.tile_pool(name="pp_s", bufs=2, space="PSUM") as pp_s, \
             tc.tile_pool(name="pp_tp", bufs=2, space="PSUM") as pp_tp, \
             tc.tile_pool(name="pp_h", bufs=4, space="PSUM") as pp_h:

            g = dict(sb=sb, s_pool=s_pool, k_pool=k_pool, h1_pool=h1_pool,
                     ht_pool=ht_pool, f_pool=f_pool, sq_pool=sq_pool,
                     nq_pool=nq_pool, pp_s=pp_s, pp_tp=pp_tp, pp_h=pp_h)

            ident = sb.tile([P, P], F32, tag="ident")
            make_identity(nc, ident)
            g["ident"] = ident
            ones1 = sb.tile([1, P], F32, tag="ones1")
            nc.vector.memset(ones1[:], 1.0)
            g["ones1"] = ones1
            alpha64 = sb.tile([H, 1], F32, tag="alpha64")
            nc.vector.memset(alpha64[:], SLOPE)
            g["alpha64"] = alpha64
            alpha128 = sb.tile([P, 1], F32, tag="alpha128")
            nc.vector.memset(alpha128[:], SLOPE)
            g["alpha128"] = alpha128
            onescol = sb.tile([H, 1], F32, tag="onescol")
            nc.vector.memset(onescol[:], 1.0)
            g["onescol"] = onescol

            # all layers' weights up front
            Ws = [
                _load_weights(nc, sb, 0, wnames["w11"], wnames["b11"],
                              wnames["w12"], wnames["b12"], wnames["w13"],
                              wnames["b13"], 3),
                _load_weights(nc, sb, 1, wnames["w21"], wnames["b21"],
                              wnames["w22"], wnames["b22"], wnames["w23"],
                              wnames["b23"], H),
                _load_weights(nc, sb, 2, wnames["w31"], wnames["b31"],
                              wnames["w32"], wnames["b32"], wnames["w33"],
                              wnames["b33"], H),
            ]

            # x2t / rb ping-pong ([H+1, N]: data rows + ones / -2sq row).
            # memset to 1.0 so row d_in serves as the ones row (never
            # rewritten; layer 1 uses row 3, layers 2/3 row 64).
            x2t_a = sb.tile([H + 1, N], F32, tag="x2t_a")
            x2t_b = sb.tile([H + 1, N], F32, tag="x2t_b")
            rb_a = sb.tile([H + 1, N], F32, tag="rb_a")
            rb_b = sb.tile([H + 1, N], F32, tag="rb_b")
            u_a = sb.tile([P, NT * H], F32, tag="u_a")
            u_b = sb.tile([P, NT * H], F32, tag="u_b")
            vbuf = sb.tile([P, NT * H], F32, tag="vbuf")
            g["obuf"] = vbuf   # final layer stages its output here
            xsb = sb.tile([P, NT * 3], F32, tag="xsb")

            for _rep in range(reps):
                nc.vector.memset(x2t_a[:], 1.0)
                nc.vector.memset(x2t_b[:], 1.0)

                # ---- init: pos -> 2*X^T rows 0..2 of x2t_a ----
                nc.sync.dma_start(xsb[:].rearrange("p (t d) -> p t d", d=3),
                                  pos_d[:].rearrange("(t p) d -> p t d", p=P))
                for t in range(NT):
                    pt = pp_tp.tile([3, P], F32, tag="tp")
                    nc.tensor.transpose(out=pt[:], in_=xsb[:, t * 3:(t + 1) * 3],
                                        identity=ident[:])
                    nc.scalar.mul(out=x2t_a[0:3, t * P:(t + 1) * P], in_=pt[:], mul=2.0)

                # ---- init: rb_a rows 0..2 + -2sq row 3; U/V for layer 1 ----
                nc.scalar.copy(out=rb_a[0:3, :], in_=x2t_a[0:3, :])
                xsq = s_pool.tile([P, N], F32, tag="s")   # borrow an S buffer
                nc.scalar.activation(out=xsq[0:3, :], in_=x2t_a[0:3, :], func=AF.Square)
                nrow = s_pool.tile([P, N], F32, tag="s")  # borrow: row 0 stages -2sq
                for c in range(N // 512):
                    ps = pp_tp.tile([1, 512], F32, tag="tp")
                    nc.tensor.matmul(out=ps[:], lhsT=onescol[0:3, :],
                                     rhs=xsq[0:3, c * 512:(c + 1) * 512],
                                     start=True, stop=True)
                    # ps holds 4*sq (x2t holds 2X); write -2sq
                    nc.scalar.activation(out=nrow[0:1, c * 512:(c + 1) * 512],
                                         in_=ps[:], func=AF.Copy, scale=-0.5)
                nc.sync.dma_start(rb_a[3:4, :], nrow[0:1, :])
                for t in range(NT):
                    _emit_uv_tile(nc, g, t, x2t_a, 3, Ws[0], u_a, vbuf)
                nc.sync.dma_start(v_ds[0][:].rearrange("(t p) f -> p t f", p=P),
                                  vbuf[:].rearrange("p (t f) -> p t f", f=H))

                _build_layer(nc, g, 3, x2t_a, rb_a, u_a, Ws[0], v_ds[0],
                             nxt=dict(x2t=x2t_b, rb=rb_b, u_sb=u_b, vbuf=vbuf,
                                      v_d=v_ds[1], W=Ws[1]))
                _build_layer(nc, g, H, x2t_b, rb_b, u_b, Ws[1], v_ds[1],
                             nxt=dict(x2t=x2t_a, rb=rb_a, u_sb=u_a, vbuf=vbuf,
                                      v_d=v_ds[2], W=Ws[2]))
                _build_layer(nc, g, H, x2t_a, rb_a, u_a, Ws[2], v_ds[2], nxt=None,
                             out_d=out_d)
    nc.finalize()
    return nc


def kernel(**inputs):
    if "nc" not in _CACHE:
        _CACHE["nc"] = build()
    nc = _CACHE["nc"]
    pos = np.ascontiguousarray(np.asarray(inputs["pos"], dtype=np.float32))
    weights = {k: np.ascontiguousarray(np.asarray(v, dtype=np.float32))
               for k, v in inputs.items() if k != "pos"}
    in_maps = []
    for b in range(B):
        m = {"pos": pos[b]}
        m.update(weights)
        in_maps.append(m)
    res = run_bass_kernel_spmd(nc, in_maps, core_ids=list(range(B)))
    out = np.stack([res.results[b]["out"] for b in range(B)], axis=0)
    return out


if __name__ == "__main__":
    rng = np.random.default_rng(0)
    fake = {"pos": rng.standard_normal((B, N, 3)).astype(np.float32)}
    for pfx in ("1", "2", "3"):
        d2 = 6 if pfx == "1" else 128
        fake[f"w{pfx}1"] = rng.standard_normal((d2, H)).astype(np.float32) * 0.2
        fake[f"w{pfx}2"] = rng.standard_normal((H, H)).astype(np.float32) * 0.12
        fake[f"w{pfx}3"] = rng.standard_normal((H, H)).astype(np.float32) * 0.12
        for j in ("1", "2", "3"):
            fake[f"b{pfx}{j}"] = np.zeros(H, np.float32)
    o = kernel(**fake)
    print("out", o.shape, o.dtype, float(np.abs(o).max()))
